# revision 5
# baseline (speedup 1.0000x reference)
"""TRN2 Bass kernel for ChemicalConvWithBonds GNN message passing.

h_out = segment_sum(silu([h[row] | h[col] | bond] @ W1 + b1) @ W2 + b2, row)

Strategy (8 NeuronCores, SPMD, v2):
  - Edges owner-sharded by destination node (row): core c owns nodes
    [c*6250, (c+1)*6250). Host sorts edges by row, splits each 128-node
    window's edges by Q-table half (int16 gather-index limit), pads to
    per-window counts shared across all cores (one SPMD graph).
  - Every core streams the FULL hT and computes the FULL Q = h@W1b table
    locally (bf16) into its own DRAM -- no AllGather. P = h@W1a + b1 for
    the local 6400 nodes stays resident in SBUF.
  - Per 128-edge tile: Q[col] rows fetched with the GPSIMD dma_gather
    ucode (int16 wrapped indices, 2048-row calls rotated across 4 SWDGE
    queues); one-hot M built on DVE (is_equal vs iota); M^T by PE
    transpose; PSUM accumulates bond@W1c + M^T-expand of P + identity-
    inject of Qg; SiLU on the scalar engine reads PSUM directly;
    scatter-add into S^T via one-hot matmul; per window
    O^T = W2-matmul(S^T) + b2 x deg, written transposed, fixed on host.
"""

import os
import sys

for p in ("/opt/trn_rl_repo", "/root/.axon_site/_ro/trn_rl_repo"):
    if os.path.isdir(p) and p not in sys.path:
        sys.path.append(p)

import contextlib
import ctypes
import types

import numpy as np
import ml_dtypes

import concourse.tile as tile
from concourse import mybir
from concourse.bacc import Bacc
from concourse.tile_rust import add_dep_helper

C = 8
H = 128
BOND = 16
G = 4
NPC = 6250
NPAD = 6400
W = 50
NCHUNK = 391          # ceil(50000/128) Q table chunks
NNODE_PAD = NCHUNK * 128   # 50048
NHALF = 196 * 128     # 25088: gather half split (int16 idx limit)
QSUB = 2048           # edges per dma_gather call
NWR = 8               # Q table written in NWR chunked stores
SCRATCH = 57344

bf = mybir.dt.bfloat16
f32 = mybir.dt.float32
i32 = mybir.dt.int32
nbf = ml_dtypes.bfloat16

LAST_EXEC_NS = None


def _install_ntff_hook():
    """Provide antenv.axon_hooks so trace=True works under axon."""
    if "antenv.axon_hooks" in sys.modules:
        return
    so_path = "/opt/axon/libaxon_pjrt.so"
    try:
        lib = ctypes.CDLL(so_path)
        lib.axon_start_nrt_profile.argtypes = [
            ctypes.POINTER(ctypes.c_int64),
            ctypes.c_size_t,
        ]
        lib.axon_start_nrt_profile.restype = ctypes.c_int64
        lib.axon_stop_nrt_profile.argtypes = [ctypes.c_char_p]
        lib.axon_stop_nrt_profile.restype = ctypes.c_int64
    except Exception:
        return

    @contextlib.contextmanager
    def _hook(output_dir, device_ids):
        import jax

        jax.devices()
        if device_ids:
            ids = (ctypes.c_int64 * len(device_ids))(*device_ids)
            rc = lib.axon_start_nrt_profile(ids, len(device_ids))
        else:
            rc = lib.axon_start_nrt_profile(None, 0)
        if rc != 0:
            raise RuntimeError(f"axon_start_nrt_profile rc={rc}")
        try:
            yield
        finally:
            n = lib.axon_stop_nrt_profile(str(output_dir).encode())
            print(f"profile: {n} file(s) -> {output_dir}", file=sys.stderr)

    mod = types.ModuleType("antenv.axon_hooks")
    mod.get_axon_ntff_profile_hook = lambda: _hook
    mod.set_axon_ntff_profile_hook = lambda h: None
    sys.modules["antenv.axon_hooks"] = mod


def host_prep(h, edge_index, bond_features, W1, b1, W2, b2):
    N, _ = h.shape

    row = np.asarray(edge_index[0], dtype=np.int64)
    col = np.asarray(edge_index[1], dtype=np.int64)
    perm = np.argsort(row, kind="stable")
    rs = row[perm]
    cs = col[perm]
    bs = np.asarray(bond_features, dtype=np.float32)[perm]

    cb = np.searchsorted(rs, np.arange(C + 1) * NPC)
    wb = np.zeros((C, W + 1), dtype=np.int64)
    for c in range(C):
        lo, hi = cb[c], cb[c + 1]
        wb[c] = lo + np.searchsorted(rs[lo:hi], c * NPC + np.arange(W + 1) * 128)

    # per (core, window): split by Q half; pad each half to shared tile counts
    cntA = np.zeros((C, W), np.int64)
    cntB = np.zeros((C, W), np.int64)
    for c in range(C):
        for w in range(W):
            lo, hi = wb[c, w], wb[c, w + 1]
            hv = cs[lo:hi] >= NHALF
            cntB[c, w] = hv.sum()
            cntA[c, w] = (hi - lo) - cntB[c, w]
    NIa = (-(-cntA.max(axis=0) // 128) * 128).astype(np.int64)
    NIb = (-(-cntB.max(axis=0) // 128) * 128).astype(np.int64)
    Tw = ((NIa + NIb) // 128).astype(np.int64)
    Tw = np.maximum(Tw, 1)
    NIa = np.where(NIa + NIb == 0, 128, NIa)
    NT = int(Tw.sum())
    t0s = np.concatenate([[0], np.cumsum(Tw)])

    lrow = np.full((C, NT * 128), 255.0, np.float32)
    bondT = np.zeros((C, BOND, NT * 128), np.float32)
    degT = np.zeros((C, 1, NPAD), np.float32)
    q16 = np.zeros((C, 128, NT * 8), np.int16)  # wrapped+replicated gather idx

    def wrap16(dst, base_col, vals):
        # idx i of this call at [i%16 (+16g), base_col + i//16]
        n = len(vals)
        blk = vals.reshape(n // 16, 16).T.astype(np.int16)
        for g in range(8):
            dst[g * 16 : (g + 1) * 16, base_col : base_col + n // 16] = blk

    for c in range(C):
        deg = np.bincount(rs[cb[c] : cb[c + 1]] - c * NPC, minlength=NPAD)
        degT[c, 0, :] = deg[:NPAD]
        for w in range(W):
            lo, hi = wb[c, w], wb[c, w + 1]
            qv = cs[lo:hi]
            hv = qv >= NHALF
            order = np.argsort(hv, kind="stable")
            na, nb = int(cntA[c, w]), int(cntB[c, w])
            s = int(t0s[w]) * 128
            # half A occupies slots [0, NIa), half B [NIa, NIa+NIb)
            posA = s + np.arange(na)
            posB = s + int(NIa[w]) + np.arange(nb)
            pos = np.concatenate([posA, posB])
            src_idx = lo + order
            lrow[c][pos] = rs[src_idx] - c * NPC - w * 128
            bondT[c][:, pos] = bs[src_idx].T
            qa = np.zeros(int(NIa[w]), np.int64)
            qa[:na] = qv[order[:na]]
            qb = np.zeros(int(NIb[w]), np.int64)
            qb[:nb] = qv[order[na:]] - NHALF
            # wrap per QSUB-sized call
            for half, arr in ((0, qa), (1, qb)):
                off = 0
                base = s if half == 0 else s + int(NIa[w])
                while off < len(arr):
                    ni = min(QSUB, len(arr) - off)
                    wrap16(q16[c], (base + off) // 16, arr[off : off + ni])
                    off += ni

    W1 = np.asarray(W1, np.float32)
    W1ab = np.ascontiguousarray(np.concatenate([W1[:H], W1[H : 2 * H]], axis=1))
    W1cT = np.asarray(W1[2 * H :], np.float32).astype(nbf)
    b1z = np.concatenate([np.asarray(b1, np.float32), np.zeros(H, np.float32)])[None]
    hT_full = np.zeros((H, NNODE_PAD), np.float32)
    hT_full[:, :N] = np.asarray(h, np.float32).T

    shared = {
        "hT": hT_full,
        "W1ab": W1ab,
        "b1z": b1z,
        "onesr": np.ones((1, H), np.float32),
        "W1cT": W1cT,
        "W2p": np.asarray(W2, np.float32).astype(nbf),
        "b2r": np.asarray(b2, np.float32)[None].astype(nbf),
        "ident": np.eye(H, dtype=np.float32).astype(nbf),
        "iotag": np.tile(np.arange(128, dtype=np.float32), (128, G)).astype(nbf),
    }
    in_maps = []
    for c in range(C):
        m = dict(shared)
        m["q16"] = q16[c]
        m["lrowp"] = np.ascontiguousarray(lrow[c].reshape(NT, 128).T).astype(nbf)
        m["bondT"] = bondT[c].astype(nbf)
        m["degT"] = degT[c].astype(nbf)
        in_maps.append(m)

    meta = {
        "Tw": [int(x) for x in Tw],
        "NIa": [int(x) for x in NIa],
        "NIb": [int(x) for x in NIb],
        "NT": NT,
    }
    return meta, in_maps


def build(meta):
    Tw = meta["Tw"]
    NIa = meta["NIa"]
    NIb = meta["NIb"]
    NT = meta["NT"]

    nc = Bacc(dynamic_dma_scratch_size=SCRATCH, num_swdge_queues=4)
    hT = nc.declare_dram_parameter("hT", [H, NNODE_PAD], f32, isOutput=False)
    W1ab = nc.declare_dram_parameter("W1ab", [H, 2 * H], f32, isOutput=False)
    b1z = nc.declare_dram_parameter("b1z", [1, 2 * H], f32, isOutput=False)
    onesr = nc.declare_dram_parameter("onesr", [1, H], f32, isOutput=False)
    W1cT = nc.declare_dram_parameter("W1cT", [BOND, H], bf, isOutput=False)
    W2p = nc.declare_dram_parameter("W2p", [H, H], bf, isOutput=False)
    b2r = nc.declare_dram_parameter("b2r", [1, H], bf, isOutput=False)
    ident = nc.declare_dram_parameter("ident", [H, H], bf, isOutput=False)
    iotag = nc.declare_dram_parameter("iotag", [128, G * 128], bf, isOutput=False)
    q16 = nc.declare_dram_parameter("q16", [128, NT * 8], mybir.dt.int16, isOutput=False)
    lrowp = nc.declare_dram_parameter("lrowp", [128, NT], bf, isOutput=False)
    bondT = nc.declare_dram_parameter("bondT", [BOND, NT * 128], bf, isOutput=False)
    degT = nc.declare_dram_parameter("degT", [1, NPAD], bf, isOutput=False)
    hTl = nc.declare_dram_parameter("hTl", [H, NPAD], f32, isOutput=False)
    outT = nc.declare_dram_parameter("outT", [H, NPAD], f32, isOutput=True)

    Q_full = nc.dram_tensor("Q_full", [NNODE_PAD, H], bf)

    SILU = mybir.ActivationFunctionType.Silu

    # Q table write chunking: NWR stores, each covering wrc[i]..wrc[i+1] chunks.
    # Boundary at chunk 196 (= NHALF) must coincide with a store boundary.
    wrc = [0, 49, 98, 147, 196, 245, 294, 343, NCHUNK]
    wmax = max(b - a for a, b in zip(wrc[:-1], wrc[1:]))

    with tile.TileContext(nc) as tc:
        with tc.tile_pool(name="cst", bufs=1) as cp:
            W1ab_sb = cp.tile([H, 2 * H], f32)
            nc.sync.dma_start(out=W1ab_sb[:], in_=W1ab[:])
            b1z_sb = cp.tile([1, 2 * H], f32)
            nc.sync.dma_start(out=b1z_sb[:], in_=b1z[:])
            ones_sb = cp.tile([1, H], f32)
            nc.sync.dma_start(out=ones_sb[:], in_=onesr[:])
            W1c_sb = cp.tile([BOND, H], bf)
            nc.sync.dma_start(out=W1c_sb[:], in_=W1cT[:])
            W2_sb = cp.tile([H, H], bf)
            nc.sync.dma_start(out=W2_sb[:], in_=W2p[:])
            b2_sb = cp.tile([1, H], bf)
            nc.sync.dma_start(out=b2_sb[:], in_=b2r[:])
            id_sb = cp.tile([H, H], bf)
            nc.sync.dma_start(out=id_sb[:], in_=ident[:])
            iota_sb = cp.tile([128, G * 128], bf)
            nc.sync.dma_start(out=iota_sb[:], in_=iotag[:])
            q16_sb = cp.tile([128, NT * 8], mybir.dt.int16)
            q16_ld = nc.sync.dma_start(out=q16_sb[:], in_=q16[:])
            lrow_sb = cp.tile([128, NT], bf)
            nc.sync.dma_start(out=lrow_sb[:], in_=lrowp[:])
            degT_sb = cp.tile([1, NPAD], bf)
            nc.sync.dma_start(out=degT_sb[:], in_=degT[:])
            P_win = cp.tile([128, W * H], bf)  # local P windows, SBUF-resident

            # ---- Phase A: full Q table (local compute, no collective) ----
            # hT streamed in 2048-col chunks; per 128-node chunk one matmul.
            HCH = 2048
            q_writes = []
            with (
                tc.tile_pool(name="pa", bufs=2) as pa,
                tc.tile_pool(name="pst", bufs=2) as pst,
                tc.tile_pool(name="pap", bufs=4, space="PSUM") as pap,
            ):
                stage = None
                si = 0  # store index
                scol = 0  # columns filled in current stage
                hT_sb = None
                for ch in range(NCHUNK):
                    if ch % (HCH // 128) == 0:
                        hT_sb = pa.tile([H, HCH], f32, tag="hTsb", name=f"hT{ch}")
                        c0 = ch * 128
                        nc.sync.dma_start(
                            out=hT_sb[:, : min(HCH, NNODE_PAD - c0)],
                            in_=hT[:, c0 : min(c0 + HCH, NNODE_PAD)],
                        )
                    if stage is None:
                        swidth = (wrc[si + 1] - wrc[si]) * 128
                        stage = pst.tile(
                            [128, wmax * 128], bf, tag="qst", name=f"qst{si}"
                        )
                        scol = 0
                    lcol = (ch % (HCH // 128)) * 128
                    ppq = pap.tile([128, H], f32, tag="ppq", name=f"ppq{ch}")
                    nc.tensor.matmul(
                        ppq[:],
                        lhsT=hT_sb[:, lcol : lcol + 128],
                        rhs=W1ab_sb[:, H : 2 * H],
                        start=True,
                        stop=True,
                    )
                    # split PSUM->stage copies across ACT and DVE
                    if ch % 2 == 0:
                        nc.scalar.copy(out=stage[:, scol : scol + 128], in_=ppq[:])
                    else:
                        nc.vector.tensor_copy(
                            out=stage[:, scol : scol + 128], in_=ppq[:]
                        )
                    scol += 128
                    if scol == swidth:
                        r0 = wrc[si] * 128
                        r1 = wrc[si + 1] * 128
                        q_writes.append(
                            nc.sync.dma_start(
                                out=Q_full[r0:r1, :].rearrange(
                                    "(k p) h -> p k h", p=128
                                ),
                                in_=stage[:, :swidth].rearrange(
                                    "p (k h) -> p k h", h=H
                                ),
                            )
                        )
                        stage = None
                        si += 1

            # ---- Phase A2: local P windows (needs per-core hT slice) ----
            # hTl = hT columns [core*NPC, core*NPC + NPAD) supplied per-core.
            with (
                tc.tile_pool(name="pb", bufs=1) as pb,
                tc.tile_pool(name="pbp", bufs=2, space="PSUM") as pbp,
            ):
                hTl_sb = pb.tile([H, NPAD], f32, tag="hTlsb", name="hTlsb")
                nc.sync.dma_start(out=hTl_sb[:], in_=hTl[:])
                for w in range(W):
                    pp = pbp.tile([128, H], f32, tag="pp", name=f"pp{w}")
                    nc.tensor.matmul(
                        pp[:],
                        lhsT=hTl_sb[:, w * 128 : (w + 1) * 128],
                        rhs=W1ab_sb[:, 0:H],
                        start=True,
                        stop=False,
                    )
                    nc.tensor.matmul(
                        pp[:],
                        lhsT=ones_sb[:],
                        rhs=b1z_sb[:, 0:H],
                        start=False,
                        stop=True,
                    )
                    nc.scalar.copy(out=P_win[:, w * H : (w + 1) * H], in_=pp[:])

            # ---- Phase B ----
            qnum = 0
            # store handles by half: half A needs stores 0..3, half B 4..7
            wrA = q_writes[0:4]
            wrB = q_writes[4:8]
            with (
                tc.tile_pool(name="gp", bufs=3) as gp,
                tc.tile_pool(name="bp", bufs=2) as bp,
                tc.tile_pool(name="sp", bufs=4) as sp,
                tc.tile_pool(name="mp", bufs=4) as mp,
                tc.tile_pool(name="tp", bufs=4) as tp,
                tc.tile_pool(name="zp", bufs=2, space="PSUM") as zp,
                tc.tile_pool(name="Tp", bufs=2, space="PSUM") as Tp,
                tc.tile_pool(name="Sp", bufs=2, space="PSUM") as Sp,
                tc.tile_pool(name="Op", bufs=1, space="PSUM") as Op,
                tc.tile_pool(name="op", bufs=3) as op,
            ):
                t0 = 0
                for w in range(W):
                    T = Tw[w]
                    bT = bp.tile([BOND, T * 128], bf, tag="bT", name=f"bT{w}")
                    nc.sync.dma_start(
                        out=bT[:], in_=bondT[:, t0 * 128 : (t0 + T) * 128]
                    )
                    # ---- Q gathers: per half, per QSUB chunk, rotating queues
                    Qg = gp.tile([128, T * 128], bf, tag="Qg", name=f"Qg{w}")
                    for half in range(2):
                        ni_h = NIa[w] if half == 0 else NIb[w]
                        base = 0 if half == 0 else NIa[w]
                        rowbase = 0 if half == 0 else NHALF
                        nrows = NHALF if half == 0 else NNODE_PAD - NHALF
                        deps = wrA if half == 0 else wrB
                        off = 0
                        while off < ni_h:
                            ni = min(QSUB, ni_h - off)
                            slot = base + off
                            gi = nc.gpsimd.dma_gather(
                                out_ap=Qg[:, slot : slot + ni].rearrange(
                                    "p (k h) -> p k h", h=H
                                ),
                                in_ap=Q_full[rowbase : rowbase + nrows, :],
                                idxs_ap=q16_sb[
                                    :,
                                    (t0 * 128 + slot) // 16 : (t0 * 128 + slot + ni)
                                    // 16,
                                ],
                                num_idxs=ni,
                                num_idxs_reg=ni,
                                elem_size=H,
                                queue_num=qnum % 4,
                                single_packet=False,
                            )
                            qnum += 1
                            for d in deps:
                                add_dep_helper(
                                    gi.ins, d.ins, sync=True, reason="Q ready"
                                )
                            add_dep_helper(
                                gi.ins, q16_ld.ins, sync=True, reason="after idx"
                            )
                            off += ni
                    pS = Sp.tile([128, 128], f32, tag="pS", name=f"pS{w}")
                    g0 = 0
                    while g0 < T:
                        gs = min(G, T - g0)
                        # one-hot M [edge, node] on DVE
                        M_sb = mp.tile([128, G * 128], bf, tag="M", name=f"M{w}_{g0}")
                        nc.vector.tensor_tensor(
                            out=M_sb[:, : gs * 128].rearrange("p (g j) -> p g j", g=gs),
                            in0=iota_sb[:, : gs * 128].rearrange(
                                "p (g j) -> p g j", g=gs
                            ),
                            in1=lrow_sb[:, t0 + g0 : t0 + g0 + gs].to_broadcast(
                                [128, gs, 128]
                            ),
                            op=mybir.AluOpType.is_equal,
                        )
                        # M^T [node, edge] via PE transpose, copied to SBUF
                        pMT = Tp.tile([128, G * 128], f32, tag="pMT", name=f"pMT{w}_{g0}")
                        for i in range(gs):
                            sl = slice(i * 128, (i + 1) * 128)
                            nc.tensor.matmul(
                                pMT[:, sl],
                                lhsT=M_sb[:, sl],
                                rhs=id_sb[:],
                                start=True,
                                stop=True,
                            )
                        MT_sb = tp.tile([128, G * 128], bf, tag="MT", name=f"MT{w}_{g0}")
                        nc.scalar.copy(out=MT_sb[:, : gs * 128], in_=pMT[:, : gs * 128])
                        # PSUM accumulate: bond@W1c + P_win-expand + Qg inject
                        pz = zp.tile([128, G * 128], f32, tag="pz", name=f"pz{w}_{g0}")
                        nc.tensor.matmul(
                            pz[:, : gs * 128],
                            lhsT=id_sb[:],
                            rhs=Qg[:, g0 * 128 : (g0 + gs) * 128],
                            start=True,
                            stop=False,
                        )
                        for i in range(gs):
                            t = g0 + i
                            sl = slice(i * 128, (i + 1) * 128)
                            esl = slice(t * 128, (t + 1) * 128)
                            nc.tensor.matmul(
                                pz[:, sl],
                                lhsT=bT[:, esl],
                                rhs=W1c_sb[:],
                                start=False,
                                stop=False,
                            )
                            nc.tensor.matmul(
                                pz[:, sl],
                                lhsT=MT_sb[:, sl],
                                rhs=P_win[:, w * H : (w + 1) * H],
                                start=False,
                                stop=True,
                            )
                        # SiLU straight from PSUM
                        s_sb = sp.tile([128, G * 128], bf, tag="s", name=f"s{w}_{g0}")
                        nc.scalar.activation(
                            out=s_sb[:, : gs * 128], in_=pz[:, : gs * 128], func=SILU
                        )
                        for i in range(gs):
                            t = g0 + i
                            sl = slice(i * 128, (i + 1) * 128)
                            nc.tensor.matmul(
                                pS[:],
                                lhsT=s_sb[:, sl],
                                rhs=M_sb[:, sl],
                                start=(t == 0),
                                stop=(t == T - 1),
                            )
                        g0 += gs
                    sT_sb = op.tile([128, 128], bf, tag="sT", name=f"sT{w}")
                    nc.scalar.copy(out=sT_sb[:], in_=pS[:])
                    pO = Op.tile([128, 128], f32, tag="pO", name=f"pO{w}")
                    nc.tensor.matmul(
                        pO[:], lhsT=W2_sb[:], rhs=sT_sb[:], start=True, stop=False
                    )
                    nc.tensor.matmul(
                        pO[:],
                        lhsT=b2_sb[:],
                        rhs=degT_sb[:, w * 128 : (w + 1) * 128],
                        start=False,
                        stop=True,
                    )
                    o_sb = op.tile([128, 128], f32, tag="o", name=f"o{w}")
                    nc.vector.tensor_copy(out=o_sb[:], in_=pO[:])
                    nc.sync.dma_start(out=outT[:, w * 128 : (w + 1) * 128], in_=o_sb[:])
                    t0 += T
    nc.finalize()
    return nc


def kernel(h, edge_index, bond_features, W1, b1, W2, b2):
    global LAST_EXEC_NS
    meta, in_maps = host_prep(h, edge_index, bond_features, W1, b1, W2, b2)
    nc = build(meta)

    hT_full = in_maps[0]["hT"]
    for c in range(C):
        in_maps[c]["hTl"] = np.ascontiguousarray(
            np.pad(
                hT_full[:, c * NPC : min((c + 1) * NPC, hT_full.shape[1])],
                ((0, 0), (0, NPAD - NPC)),
            )
        )

    from concourse.bass_utils import run_bass_kernel_spmd

    trace = os.environ.get("GNN_KERNEL_TRACE", "0") == "1"
    if trace:
        _install_ntff_hook()
    res = run_bass_kernel_spmd(nc, in_maps, list(range(C)), trace=trace)
    LAST_EXEC_NS = res.exec_time_ns

    outs = []
    for c in range(C):
        o = np.asarray(res.results[c]["outT"], dtype=np.float32)
        outs.append(o.T[:NPC])
    return np.ascontiguousarray(np.concatenate(outs, axis=0))
